# revision 4
# baseline (speedup 1.0000x reference)
"""Trainium2 Bass kernel for nn_EnsembleParallelQSG.

Computation (see reference): per (b, s) token, softmax over V per path with
per-path temperature; cross-path cosine similarity of the softmax vectors;
consistency -> prior-weighted softmax over paths -> path weights; weighted
fusion of raw logits over paths; argmax over V; plus tiny per-batch stats.

Key algebraic simplification: cosine similarity is invariant to per-vector
positive scaling, so cos(softmax(s_p), softmax(s_q)) == cos(e_p, e_q) with
e_p = exp(s_p).  We therefore never need the softmax normalizers, only
  G_pq = sum_v e_p e_q   (6 off-diagonal pairs, via DVE scalar_tensor_tensor
                          with fused accumulate)
  n_p^2 = sum_v exp(2 s_p)  (diagonal, free via ACT activation accum_out)
The [P] parameter vectors are folded into immediates at trace time.

Sharding: 1024 (b, s) tokens split contiguously across 8 cores (data
parallel, 128 tokens per core = the SBUF partition dim).  No cross-core
communication.
"""

import sys

sys.path.insert(0, "/opt/trn_rl_repo")

import numpy as np

B, P, S, V = 2, 4, 512, 32000
N_CORES = 8
TOK = (B * S) // N_CORES           # 128 tokens per core
CHUNK = 2000
NCH = V // CHUNK                   # 16
PAIRS = [(0, 1), (0, 2), (0, 3), (1, 2), (1, 3), (2, 3)]
# pair indices contributing to each path's consistency row-sum
ROWS = [[0, 1, 2], [0, 3, 4], [1, 3, 5], [2, 4, 5]]

_PROGRAM_CACHE = {}


def _patch_tile_drain():
    """This container's walrus only accepts one sync-wait on the Tile exit
    Drain; split the extras into standalone wait-nops (SP executes its
    stream in order, so semantics are unchanged)."""
    import concourse.mybir as mybir
    from concourse import tile as tile_mod
    from concourse.vector_clock import ScopedClock

    if getattr(tile_mod.TileContext, "_drain_patched", False):
        return

    def _drain_and_barrier(self, tick_clock, wait_clock):
        nc = self.nc
        drain_inst = nc.sync.drain()
        wait_clock.add_sem_waits(
            drain_inst.ins, ScopedClock({None: tick_clock.global_clock})
        )
        si = drain_inst.ins.sync_info
        if si is not None and si.on_wait and len(si.on_wait) > 1:
            waits = list(si.on_wait)
            si.on_wait = waits[:1]
            for w in waits[1:]:
                nop = nc.sync.nop()
                nop.ins.sync_info = mybir.SyncInfo(on_wait=[w], on_update=[])
        nc.all_engine_barrier()
        popped = nc._tile_sem_poison_stack.pop()
        assert popped is self._sem_poison
        nc.clear_and_free_semaphores(list(self.sems.allocated().values()))
        nc.all_engine_barrier()

    tile_mod.TileContext._drain_and_barrier = _drain_and_barrier
    tile_mod.TileContext._drain_patched = True


def _split_excess_waits(nc):
    """This container's walrus accepts only ONE sync-wait per instruction.
    Tile can attach several.  Split the extras onto same-engine no-ops
    inserted immediately before the instruction (the engine executes its
    stream in order, so wait-then-wait-then-op is equivalent)."""
    import concourse.mybir as mybir

    nop_cls = None
    counter = [0]
    for f in nc.m.functions:
        for bb in f.blocks:
            out = []
            changed = False
            for ins in bb.instructions:
                si = ins.sync_info
                if si is not None and si.on_wait and len(si.on_wait) > 1:
                    if nop_cls is None:
                        nop_cls = _get_noop_cls()
                    waits = list(si.on_wait)
                    si.on_wait = waits[-1:]
                    for w in waits[:-1]:
                        counter[0] += 1
                        nop = nop_cls(name=f"wsplit-{counter[0]}")
                        nop.engine = ins.engine
                        nop.sync_info = mybir.SyncInfo(on_wait=[w], on_update=[])
                        out.append(nop)
                    changed = True
                out.append(ins)
            if changed:
                bb.instructions = out


def _get_noop_cls():
    import bass_rust
    return bass_rust.InstNoOp


def _build_program(inv_temps, prior, repeat=1):
    """Trace the per-core Bass program.  inv_temps / prior are baked in as
    immediates (they come from the tiny [P] parameter vectors)."""
    import concourse.bass as bass
    import concourse.mybir as mybir
    from concourse.tile import TileContext

    _patch_tile_drain()
    f32 = mybir.dt.float32
    Alu = mybir.AluOpType
    Act = mybir.ActivationFunctionType

    nc = bass.Bass()
    x_ext = nc.declare_dram_parameter("x", [P, TOK, V], f32, isOutput=False)
    ens_ext = nc.declare_dram_parameter("ens", [TOK, V], f32, isOutput=True)
    w_ext = nc.declare_dram_parameter("w", [TOK, P], f32, isOutput=True)
    cmax_ext = nc.declare_dram_parameter("cmax", [TOK, NCH], f32, isOutput=True)

    with TileContext(nc) as tc:
        with (
            tc.tile_pool(name="xp", bufs=8) as xp,
            tc.tile_pool(name="ep", bufs=6) as ep,
            tc.tile_pool(name="ascr", bufs=2) as ascr_p,
            tc.tile_pool(name="dscr", bufs=2) as dscr_p,
            tc.tile_pool(name="accp", bufs=5) as accp,
            tc.tile_pool(name="small", bufs=1) as small,
        ):
          for _rep in range(repeat):
            gram_cols = small.tile([TOK, 6 * NCH], f32, tag="gramc")
            nsq_cols = small.tile([TOK, 4 * NCH], f32, tag="nsqc")

            # ---------------- pass 1: Gram + norms ----------------
            for c in range(NCH):
                v0 = c * CHUNK
                es = []
                for p in range(P):
                    x_t = xp.tile([TOK, CHUNK], f32, tag="x")
                    nc.sync.dma_start(out=x_t[:], in_=x_ext[p, :, v0:v0 + CHUNK])
                    e_t = ep.tile([TOK, CHUNK], f32, tag="e")
                    nc.scalar.activation(e_t[:], x_t[:], Act.Exp,
                                         scale=float(inv_temps[p]))
                    ascr = ascr_p.tile([TOK, CHUNK], f32, tag="as")
                    nc.scalar.activation(
                        ascr[:], x_t[:], Act.Exp,
                        scale=float(2.0 * inv_temps[p]),
                        accum_out=nsq_cols[:, p * NCH + c:p * NCH + c + 1])
                    es.append(e_t)
                for j, (p, q) in enumerate(PAIRS):
                    dscr = dscr_p.tile([TOK, CHUNK], f32, tag="ds")
                    nc.vector.scalar_tensor_tensor(
                        out=dscr[:], in0=es[p][:], scalar=1.0, in1=es[q][:],
                        op0=Alu.mult, op1=Alu.mult,
                        accum_out=gram_cols[:, j * NCH + c:j * NCH + c + 1])

            # ---------------- mid: per-token weights ----------------
            G6 = small.tile([TOK, 6], f32, tag="G6")
            nc.vector.reduce_sum(
                out=G6[:], in_=gram_cols[:].rearrange("t (j c) -> t j c", c=NCH),
                axis=mybir.AxisListType.X)
            N4 = small.tile([TOK, 4], f32, tag="N4")
            nc.vector.reduce_sum(
                out=N4[:], in_=nsq_cols[:].rearrange("t (p c) -> t p c", c=NCH),
                axis=mybir.AxisListType.X)
            # inv_n = exp(-0.5 * ln(n^2))  (Ln/Exp share one ACT table set)
            L4 = small.tile([TOK, 4], f32, tag="L4")
            nc.scalar.activation(L4[:], N4[:], Act.Ln)
            invn = small.tile([TOK, 4], f32, tag="invn")
            nc.scalar.activation(invn[:], L4[:], Act.Exp, scale=-0.5)
            # sim_pq = G_pq * invn_p * invn_q
            ip6 = small.tile([TOK, 6], f32, tag="ip6")
            iq6 = small.tile([TOK, 6], f32, tag="iq6")
            for j, (p, q) in enumerate(PAIRS):
                nc.vector.tensor_copy(out=ip6[:, j:j + 1], in_=invn[:, p:p + 1])
                nc.vector.tensor_copy(out=iq6[:, j:j + 1], in_=invn[:, q:q + 1])
            s6 = small.tile([TOK, 6], f32, tag="s6")
            nc.vector.tensor_mul(out=s6[:], in0=G6[:], in1=ip6[:])
            sim6 = small.tile([TOK, 6], f32, tag="sim6")
            nc.vector.tensor_mul(out=sim6[:], in0=s6[:], in1=iq6[:])
            # weighted consistency = (sum_{q!=p} sim) / (P-1) * prior_p
            cs = small.tile([TOK, 4], f32, tag="cs")
            wc = small.tile([TOK, 4], f32, tag="wc")
            for p in range(P):
                a, b_, cc = ROWS[p]
                nc.vector.tensor_add(out=cs[:, p:p + 1], in0=sim6[:, a:a + 1],
                                     in1=sim6[:, b_:b_ + 1])
                nc.vector.tensor_add(out=cs[:, p:p + 1], in0=cs[:, p:p + 1],
                                     in1=sim6[:, cc:cc + 1])
                nc.vector.tensor_scalar_mul(wc[:, p:p + 1], cs[:, p:p + 1],
                                            float(prior[p] / (P - 1)))
            # softmax over the 4 path columns
            m1 = small.tile([TOK, 1], f32, tag="m1")
            nc.vector.reduce_max(out=m1[:], in_=wc[:], axis=mybir.AxisListType.X)
            d4 = small.tile([TOK, 4], f32, tag="d4")
            nc.vector.tensor_scalar(out=d4[:], in0=wc[:], scalar1=m1[:],
                                    scalar2=None, op0=Alu.subtract)
            e4 = small.tile([TOK, 4], f32, tag="e4")
            ssum = small.tile([TOK, 1], f32, tag="ssum")
            nc.scalar.activation(e4[:], d4[:], Act.Exp, accum_out=ssum[:])
            rs = small.tile([TOK, 1], f32, tag="rs")
            nc.vector.reciprocal(out=rs[:], in_=ssum[:])
            w4 = small.tile([TOK, 4], f32, tag="w4")
            nc.vector.tensor_scalar(out=w4[:], in0=e4[:], scalar1=rs[:],
                                    scalar2=None, op0=Alu.mult)
            nc.sync.dma_start(out=w_ext[:], in_=w4[:])

            # ---------------- pass 2: fusion + chunk max ----------------
            cmax_sb = small.tile([TOK, NCH], f32, tag="cmaxsb")
            for c in range(NCH):
                v0 = c * CHUNK
                xs = []
                for p in range(P):
                    x_t = xp.tile([TOK, CHUNK], f32, tag="x")
                    nc.sync.dma_start(out=x_t[:], in_=x_ext[p, :, v0:v0 + CHUNK])
                    xs.append(x_t)
                a0 = accp.tile([TOK, CHUNK], f32, tag="acc")
                nc.scalar.activation(a0[:], xs[0][:], Act.Copy, scale=w4[:, 0:1])
                a1 = accp.tile([TOK, CHUNK], f32, tag="acc")
                nc.vector.scalar_tensor_tensor(
                    out=a1[:], in0=xs[1][:], scalar=w4[:, 1:2], in1=a0[:],
                    op0=Alu.mult, op1=Alu.add)
                a2 = accp.tile([TOK, CHUNK], f32, tag="acc")
                nc.vector.scalar_tensor_tensor(
                    out=a2[:], in0=xs[2][:], scalar=w4[:, 2:3], in1=a1[:],
                    op0=Alu.mult, op1=Alu.add)
                ens_t = accp.tile([TOK, CHUNK], f32, tag="acc")
                nc.vector.scalar_tensor_tensor(
                    out=ens_t[:], in0=xs[3][:], scalar=w4[:, 3:4], in1=a2[:],
                    op0=Alu.mult, op1=Alu.add)
                m8 = small.tile([TOK, 8], f32, tag=f"m8_{c % 2}")
                nc.vector.max(out=m8[:], in_=ens_t[:])
                nc.vector.tensor_copy(out=cmax_sb[:, c:c + 1], in_=m8[:, 0:1])
                nc.sync.dma_start(out=ens_ext[:, v0:v0 + CHUNK], in_=ens_t[:])
            nc.sync.dma_start(out=cmax_ext[:], in_=cmax_sb[:])

    _split_excess_waits(nc)
    return nc


def _get_program(inv_temps, prior):
    key = (tuple(float(v) for v in inv_temps), tuple(float(v) for v in prior))
    if key not in _PROGRAM_CACHE:
        _PROGRAM_CACHE[key] = _build_program(inv_temps, prior)
    return _PROGRAM_CACHE[key]


def run(path_logits, path_temperatures, path_prior, trace=False):
    from concourse.bass_utils import run_bass_kernel_spmd

    path_logits = np.ascontiguousarray(np.asarray(path_logits, dtype=np.float32))
    temps = np.abs(np.asarray(path_temperatures, dtype=np.float64)) + 0.1
    inv_temps = 1.0 / temps
    pr = np.asarray(path_prior, dtype=np.float64)
    pe = np.exp(pr - pr.max())
    prior = pe / pe.sum()

    nc = _get_program(inv_temps, prior)

    flat = path_logits.transpose(0, 2, 1, 3).reshape(B * S, P, V)  # [tok, P, V]
    in_maps = []
    for k in range(N_CORES):
        sl = flat[k * TOK:(k + 1) * TOK]                  # [TOK, P, V]
        in_maps.append({"x": np.ascontiguousarray(sl.transpose(1, 0, 2))})

    res = run_bass_kernel_spmd(nc, in_maps, list(range(N_CORES)), trace=trace)

    ens = np.empty((B * S, V), dtype=np.float32)
    w_all = np.empty((B * S, P), dtype=np.float32)
    tokens = np.empty((B * S,), dtype=np.int32)
    for k in range(N_CORES):
        r = res.results[k]
        t0 = k * TOK
        ens[t0:t0 + TOK] = r["ens"]
        w_all[t0:t0 + TOK] = r["w"]
        cmax = r["cmax"]                                   # [TOK, NCH]
        cstar = np.argmax(cmax, axis=1)                    # first max chunk
        seg = r["ens"].reshape(TOK, NCH, CHUNK)[np.arange(TOK), cstar]
        tokens[t0:t0 + TOK] = (cstar * CHUNK
                               + np.argmax(seg, axis=1)).astype(np.int32)

    ensemble_logits = ens.reshape(B, S, V)
    tokens = tokens.reshape(B, S)
    path_weights_avg = w_all.reshape(B, S, P).mean(axis=1, dtype=np.float64)
    entropy = -np.sum(path_weights_avg * np.log(path_weights_avg + 1e-8), axis=-1)
    consistency_score = (1.0 - entropy / np.log(P)).astype(np.float32)
    path_weights_avg = path_weights_avg.astype(np.float32)
    return (tokens, ensemble_logits, path_weights_avg, consistency_score), res


def kernel(path_logits, path_temperatures, path_prior):
    out, _ = run(path_logits, path_temperatures, path_prior, trace=False)
    return out
